# revision 17
# baseline (speedup 1.0000x reference)
"""AttentivePooling Trainium2 kernel (streaming, bf16 score path).

Reference semantics (h_all: [T, B, D] f32, xin unused):
    h_last = h_all[-1]                       # [B, D]
    a[b, t] = <h_all[t, b, :], h_last[b, :]> / sqrt(D)
    r = relu(a)
    w = r / (sum_t r + 1e-9)
    out[b, d] = sum_t w[b, t] * h_all[t, b, d]

Because normalization happens after the relu, out = num / (Z + eps) with
num[b] = sum_t relu(a[b,t]) h[t,b] and Z[b] = sum_t relu(a[b,t]) -- both
accumulate chunk-by-chunk, so one streaming pass over h suffices.

Strategy: data-parallel over B across 8 cores (8 batches/core).  Per core,
stream 16 T-chunks of [128(t), 8(b)*512(d)]:
  - each chunk is ONE fully contiguous 2MB HWDGE DMA (128 rows x 16KB)
    issued from the Sync engine so loads never wait on compute engines;
    6 chunk buffers keep the DMA queues saturated.
  - score multiplies read h as bf16 via a stride-2 view of the f32 data
    (the top 2 bytes of an f32 are its truncated bf16), halving SBUF read
    traffic; h_last is pre-converted to packed bf16 with the 1/sqrt(D)
    scale folded in, one private copy for DVE and one for GPSIMD (a
    shared copy measurably slows both engines).
  - multiplies: 5 batches on DVE, 3 on GPSIMD; reductions of the bf16
    products: 6 on ACT (accum_out), 2 on DVE (tensor_reduce).
  - one ACT relu per chunk produces w [128, 8] (f32r); a DVE add
    accumulates w into wacc for the Z computation.
  - PE accumulates num[b] into 8 per-batch [1, 512] PSUM banks (full
    fp32 h as rhs, f32r fast path).
  - epilogue: GPSIMD partition_all_reduce gives Z; DVE computes
    1/(Z+eps); ACT scales each pooled row into quadrant-aligned result
    rows; two strided 8KB stores.
"""

import numpy as np
from contextlib import ExitStack

import concourse.bass as bass
import concourse.tile as tile
from concourse import bacc, mybir
from concourse.bass_utils import run_bass_kernel_spmd

T, B, D = 2048, 64, 512
NCORES = 8
BPC = B // NCORES  # batches per core
P = 128
TC = T // P  # 16 T-chunks
BD = BPC * D  # 4096
SCALE = float(1.0 / np.sqrt(np.float32(D)))
MULT_GP = (4, 5, 6, 7)  # GPSIMD multiply + ACT reduce; rest fused on DVE
NBUF = 6  # chunk buffers in flight

_nc_cache = None


def _build():
    global _nc_cache
    if _nc_cache is not None:
        return _nc_cache
    nc = bacc.Bacc("TRN2", debug=False, target_bir_lowering=False, num_devices=NCORES)
    h = nc.dram_tensor("h", [T, BPC, D], mybir.dt.float32r, kind="ExternalInput")
    out = nc.dram_tensor("out", [BPC, D], mybir.dt.float32, kind="ExternalOutput")
    h_ap = h.ap()
    out_ap = out.ap()
    f32 = mybir.dt.float32
    f32r = mybir.dt.float32r
    bf16 = mybir.dt.bfloat16

    with tile.TileContext(nc) as tc:
        with ExitStack() as ctx:
            hpool = ctx.enter_context(tc.tile_pool(name="h", bufs=NBUF))
            tmpp = ctx.enter_context(tc.tile_pool(name="tmp", bufs=4))
            tmpg = ctx.enter_context(tc.tile_pool(name="tmpg", bufs=3))
            scwp = ctx.enter_context(tc.tile_pool(name="scw", bufs=3))
            constp = ctx.enter_context(tc.tile_pool(name="const", bufs=1))
            psp = ctx.enter_context(tc.tile_pool(name="ps", bufs=1, space="PSUM"))

            # per-partition running sum of relu'd scores (one col per batch);
            # reduced across partitions once at the end on GPSIMD
            wacc = constp.tile([P, BPC], f32, name="wacc")
            nc.vector.memset(wacc[:], 0.0)

            # h_last broadcast to all partitions straight from DRAM, then
            # converted to packed bf16 with SCALE folded in -- one private
            # copy per multiplying engine
            hl_f32 = constp.tile([P, BD], f32, name="hl_f32")
            src_bc = (
                h_ap[T - 1 : T, :, :]
                .bitcast(f32)
                .rearrange("p b d -> p (b d)")
                .broadcast_to([P, BD])
            )
            nc.sync.dma_start(hl_f32[:], src_bc)
            hl_v = constp.tile([P, BD], bf16, name="hl_v")
            nc.vector.tensor_scalar_mul(hl_v[:], hl_f32[:], SCALE)
            hl_g = constp.tile([P, BD], bf16, name="hl_g")
            nc.scalar.activation(
                hl_g[:], hl_f32[:], mybir.ActivationFunctionType.Copy, scale=SCALE
            )

            # persistent accumulators: one PSUM bank per batch (matmul
            # outputs with K=128 must start at partition 0)
            pouts = [psp.tile([1, D], f32, name=f"pout{b}") for b in range(BPC)]

            # ACT writes need quadrant-aligned partition offsets: result
            # rows live at partitions {0,32,64,96} of two tiles.
            res = [constp.tile([P, D], f32, name=f"res{i}") for i in range(2)]
            zeps = constp.tile([1, BPC], f32, name="zeps")
            zrec = constp.tile([1, BPC], f32, name="zrec")

            hc_tiles = {}

            def load(c):
                t = hpool.tile([P, BPC, D], f32r, tag="hc", name="h_sb")
                nc.sync.dma_start(t[:], h_ap[c * P : (c + 1) * P, :, :])
                hc_tiles[c] = t

            for c in range(min(NBUF - 1, TC)):
                load(c)

            for c in range(TC):
                hc = hc_tiles.pop(c)
                scr = scwp.tile([P, BPC], f32, tag="scr")
                w = scwp.tile([P, BPC], f32r, tag="w")

                gp_prods = {}
                for b in range(BPC):
                    hc_hi = hc[:, b, :].bitcast(bf16)[:, 1::2]
                    if b in MULT_GP:
                        prod = tmpg.tile([P, D], bf16, tag="tg")
                        nc.gpsimd.tensor_tensor(
                            prod[:], hc_hi, hl_g[:, b * D : (b + 1) * D],
                            mybir.AluOpType.mult,
                        )
                        gp_prods[b] = prod
                    else:
                        # fused multiply + row-sum in one DVE pass
                        prod = tmpp.tile([P, D], bf16, tag="tv")
                        nc.vector.scalar_tensor_tensor(
                            prod[:],
                            hc_hi,
                            1.0,
                            hl_v[:, b * D : (b + 1) * D],
                            mybir.AluOpType.mult,
                            mybir.AluOpType.mult,
                            accum_out=scr[:, b : b + 1],
                        )

                for b, prod in gp_prods.items():
                    nc.scalar.activation(
                        prod[:],
                        prod[:],
                        mybir.ActivationFunctionType.Copy,
                        accum_out=scr[:, b : b + 1],
                    )

                nc.scalar.activation(w[:], scr[:], mybir.ActivationFunctionType.Relu)
                nc.vector.tensor_tensor(
                    wacc[:], wacc[:], w[:].bitcast(f32), mybir.AluOpType.add
                )

                if c + NBUF - 1 < TC:
                    load(c + NBUF - 1)

                for b in range(BPC):
                    nc.tensor.matmul(
                        pouts[b][:],
                        w[:, b : b + 1],
                        hc[:, b, :],
                        start=(c == 0),
                        stop=(c == TC - 1),
                    )

            zred = constp.tile([P, BPC], f32, name="zred")
            nc.gpsimd.partition_all_reduce(
                zred[:], wacc[:], channels=P, reduce_op=bass.bass_isa.ReduceOp.add
            )
            nc.vector.tensor_scalar_add(zeps[:], zred[0:1, :], 1e-9)
            nc.vector.reciprocal(zrec[:], zeps[:])
            for b in range(BPC):
                rt, rrow = res[b // 4], (b % 4) * 32
                nc.scalar.mul(
                    rt[rrow : rrow + 1, :], pouts[b][:], zrec[0:1, b : b + 1]
                )
            nc.sync.dma_start(out_ap[0:4, :], res[0][0:P:32, :])
            nc.sync.dma_start(out_ap[4:8, :], res[1][0:P:32, :])

    nc.finalize()
    _nc_cache = nc
    return nc


def _run(h_all: np.ndarray, trace: bool = False):
    nc = _build()
    h_all = np.ascontiguousarray(np.asarray(h_all), dtype=np.float32)
    assert h_all.shape == (T, B, D)
    in_maps = [
        {"h": np.ascontiguousarray(h_all[:, c * BPC : (c + 1) * BPC, :])}
        for c in range(NCORES)
    ]
    r = run_bass_kernel_spmd(nc, in_maps, list(range(NCORES)), trace=trace)
    out = np.concatenate([r.results[c]["out"] for c in range(NCORES)], axis=0)
    return out, r


def kernel(h_all: np.ndarray, xin: np.ndarray | None = None) -> np.ndarray:
    out, _ = _run(h_all)
    return out
